# revision 37
# baseline (speedup 1.0000x reference)
"""Trainium2 Bass kernel for batched multi-head attention.

Full module:  out = softmax((X_q Wq)(X_k Wk)^T / sqrt(dh) + keymask) (X_v Wv) * qmask
Shapes: B=4, S=2048, D=1024, H=16, dh=64.

Sharding over 8 NeuronCores: core c -> (batch b = c//2, head-group g = c%2).
Each core computes batch b, heads g*8..g*8+8 (Wq/Wk/Wv column-sharded by head).
No collectives; the host scatters inputs and gathers the [2048, 512] output
blocks into the full [4, 2048, 1024] output.

Per-core dataflow (all matmuls in float32r -> full PE rate at N>=256):
  1. PE-transpose X_q/X_k/X_v tiles to X^T (contraction dim on partitions).
  2. Projections: QW^T,KW^T = (W chunks)^T stationary x X^T moving -> [m, s];
     VW = (X^T chunks) stationary x W moving -> [s, m] (natural), stored with
     a ones-column appended per head for free softmax denominators.
  3. Per head h, q-half qh (softmax-pipelined over 16 k-chunks):
       S^T(kc) = KW^T_chunk^T @ QW^T      (K=64 matmul, auto 64x128 array tile)
       P(kc)   = exp(S^T * 0.125 + vbias) (ScalarE, mask+scale fused)
       O^T    += [VW|1]^T @ P(kc)         (K=128, accumulated in PSUM)
     Then PE-transpose the [65, q] O^T block (row 64 = sum of exp), and
     normalize out = O * (qmask/denom) on VectorE.
"""

import os
import sys
import time
import threading

for _p in ("/opt/trn_rl_repo", "/opt/pypackages"):
    if _p not in sys.path and os.path.isdir(_p):
        sys.path.append(_p)

import numpy as np
from contextlib import ExitStack

import concourse.bass as bass
import concourse.tile as tile
from concourse import bacc, mybir
from concourse.bass_utils import run_bass_kernel_spmd
from concourse.masks import make_identity

B, S, D = 4, 2048, 1024
HEADS, DH = 16, 64
NEG_BIG = 1e10
N_CORES = 8
HG = HEADS // 2          # 8 heads per core
MC = HG * DH             # 512 output cols per core
NSC = S // 128           # 16 seq chunks
NDC = D // 128           # 8 contraction chunks
NMC = MC // 128          # 4 head-dim chunks (of this core's 512 cols)
NKC = NSC                # 16 key chunks
NQH = 2                  # q halves
QH = S // NQH            # 1024

F32 = mybir.dt.float32
F32R = mybir.dt.float32r
EXP = mybir.ActivationFunctionType.Exp

# "k128": AV as one K=128 matmul (array mode switches 64<->128 per k-chunk)
# "k64" : AV split into two K=64 matmuls on array tiles (0,0)/(64,0) -> the
#         whole attention loop stays in 64x128 row-tiled mode.
AV_MODE = os.environ.get("AV_MODE", "k64")
N_FILLER = int(os.environ.get("N_FILLER", "0"))


def _r(ap):
    """reinterpret an fp32 AP as float32r for full-rate PE matmul"""
    return ap.bitcast(F32R)


def _emit(tc, t):
    nc = tc.nc
    ctx = ExitStack()

    # ---------------- persistent pools ----------------
    cpool = ctx.enter_context(tc.tile_pool(name="const", bufs=1))
    # prefetch the first X tiles before anything else so the transpose
    # pipeline starts as early as possible
    xq_dram = t["xq"].ap().rearrange("(sc p) d -> sc p d", p=128)
    pre_pool = ctx.enter_context(tc.tile_pool(name="pre", bufs=1))
    pre_x = []
    for i in range(6):
        xpre = pre_pool.tile([128, D], F32R, name=f"xpre{i}", tag=f"xpre{i}")
        nc.sync.dma_start(xpre[:], xq_dram[i])
        pre_x.append(xpre)

    ident = cpool.tile([128, 128], F32)
    make_identity(nc, ident[:])
    ident_r = cpool.tile([128, 128], F32R)
    nc.vector.tensor_copy(ident_r[:], ident[:])
    vbias = cpool.tile([128, NKC], F32)
    nc.sync.dma_start(vbias[:], t["vbias"].ap())
    qmaskT = cpool.tile([128, NSC], F32)
    nc.sync.dma_start(qmaskT[:], t["qmaskT"].ap())

    scratch_bf = cpool.tile([128, 128], mybir.dt.bfloat16)
    nc.vector.memset(scratch_bf[:], 0.0)

    qk_pool = ctx.enter_context(tc.tile_pool(name="qk", bufs=1))
    qwT = qk_pool.tile([128, NMC, S], F32R)        # [m%128, mc, s] 32KB/part
    kwT = qk_pool.tile([128, NMC, S], F32R)
    vw = qk_pool.tile([128, NKC, HG, DH + 1], F32R)  # [k%128, kc, h, dh|1]
    ones = cpool.tile([128, 1], F32)
    nc.vector.memset(ones[:], 1.0)
    nc.vector.tensor_copy(                           # denominator ones column
        vw[:, :, :, DH:DH + 1], ones[:].broadcast_to([128, NKC, HG, 1])
    )

    # ---------------- projection phase ----------------
    pctx = ExitStack()
    xt_pool = pctx.enter_context(tc.tile_pool(name="xt", bufs=1))
    x_pool = pctx.enter_context(tc.tile_pool(name="x", bufs=4))
    w_pool = pctx.enter_context(tc.tile_pool(name="w", bufs=2))
    psum_t = pctx.enter_context(tc.tile_pool(name="ps_t", bufs=2, space="PSUM"))
    psum_p = pctx.enter_context(tc.tile_pool(name="ps_p", bufs=2, space="PSUM"))

    HSC = NSC // 2  # s-chunks per half

    for xi, (xname, kind) in enumerate((("xq", "q"), ("xk", "k"), ("xv", "v"))):
        x_dram = t[xname].ap().rearrange("(sc p) d -> sc p d", p=128)
        w_dram = t["w" + kind].ap().rearrange("(dc p) m -> p dc m", p=128)
        w_sb = w_pool.tile([128, NDC, MC], F32R, tag="w")
        nc.sync.dma_start(w_sb[:], w_dram)

        for sh in range(2):  # s-halves
            # transpose this half of X into xt [d%128, dc, s_local]
            xt = xt_pool.tile([128, NDC, QH], F32R, tag="xt")
            for scl in range(HSC):
                sc = sh * HSC + scl
                if xname == "xq" and sh == 0 and scl < len(pre_x):
                    xt_in = pre_x[scl]
                else:
                    xt_in = x_pool.tile([128, D], F32R, tag="x")
                    nc.sync.dma_start(xt_in[:], x_dram[sc])
                pt = psum_t.tile([128, NDC, 128], F32R, tag="pt")
                for dc in range(NDC):
                    nc.tensor.transpose(
                        pt[:, dc, :], xt_in[:, dc * 128:(dc + 1) * 128], ident_r[:]
                    )
                if scl % 2 == 0:
                    nc.vector.tensor_copy(xt[:, :, scl * 128:(scl + 1) * 128], pt[:])
                else:
                    nc.scalar.copy(xt[:, :, scl * 128:(scl + 1) * 128], pt[:])

            if kind in ("q", "k"):
                dst = qwT if kind == "q" else kwT
                for mc in range(NMC):
                    pp = psum_p.tile([128, QH], F32, tag="pp")
                    for dc in range(NDC):
                        for nh in range(QH // 512):
                            nc.tensor.matmul(
                                pp[:, nh * 512:(nh + 1) * 512],
                                w_sb[:, dc, mc * 128:(mc + 1) * 128],
                                xt[:, dc, nh * 512:(nh + 1) * 512],
                                start=(dc == 0),
                                stop=(dc == NDC - 1),
                            )
                    nc.vector.tensor_copy(
                        dst[:, mc, sh * QH:(sh + 1) * QH], pp[:]
                    )
            else:
                for scl in range(HSC):
                    sc = sh * HSC + scl
                    pv = psum_p.tile([128, MC], F32, tag="pp")
                    for dc in range(NDC):
                        nc.tensor.matmul(
                            pv[:],
                            xt[:, dc, scl * 128:(scl + 1) * 128],
                            w_sb[:, dc, :],
                            start=(dc == 0),
                            stop=(dc == NDC - 1),
                        )
                    nc.vector.tensor_copy(
                        vw[:, sc, :, 0:DH],
                        pv[:].rearrange("p (h d) -> p h d", h=HG),
                    )

    pctx.close()

    # ---------------- attention phase ----------------
    actx = ExitStack()
    p_pool = actx.enter_context(tc.tile_pool(name="p", bufs=5))
    ot_pool = actx.enter_context(tc.tile_pool(name="ot", bufs=3))
    rq_pool = actx.enter_context(tc.tile_pool(name="rq", bufs=2))
    out_pool = actx.enter_context(tc.tile_pool(name="out", bufs=4))
    psum_s = actx.enter_context(tc.tile_pool(name="ps_s", bufs=2, space="PSUM"))
    psum_o = actx.enter_context(tc.tile_pool(name="ps_o", bufs=2, space="PSUM"))

    # DRAM view: [qh, p, qb, h, d] for per-(head, q-half) strip stores
    out_v = t["out"].ap().rearrange(
        "(a qb p) (hh d) -> a p qb hh d", a=NQH, p=128, hh=HG
    )

    def filler(n):
        for _ in range(n):
            nc.tensor.ldweights(scratch_bf[:])

    pending_tail = [None]

    for h in range(HG):
        mc_h = h // 2
        p0 = (h % 2) * 64
        kw_h = kwT[p0:p0 + 64, mc_h, :]
        qw_h = qwT[p0:p0 + 64, mc_h, :]
        for qh in range(NQH):
            q0 = qh * QH
            o_ps = psum_o.tile([DH + 1, QH], F32, tag="o")
            if AV_MODE == "k64":
                o_hi = psum_o.tile([DH + 1, QH], F32, tag="o")
            else:
                o_hi = None

            def emit_s(kc):
                s_ps = psum_s.tile([128, QH], F32, tag="s")
                for nh in range(QH // 512):
                    nc.tensor.matmul(
                        s_ps[:, nh * 512:(nh + 1) * 512],
                        kw_h[:, kc * 128:(kc + 1) * 128],
                        qw_h[:, q0 + nh * 512:q0 + (nh + 1) * 512],
                        start=True,
                        stop=True,
                    )
                return s_ps

            def emit_exp(kc, s_ps):
                p_t = p_pool.tile([128, QH], F32R, tag="p")
                nc.scalar.activation(
                    p_t[:], s_ps[:], EXP,
                    bias=vbias[:, kc:kc + 1], scale=0.125,
                )
                return p_t

            def emit_av(kc, p_t):
                first, last = kc == 0, kc == NKC - 1
                for nh in range(QH // 512):
                    osl = o_ps[:, nh * 512:(nh + 1) * 512]
                    psl = p_t[:, nh * 512:(nh + 1) * 512]
                    if AV_MODE == "k128":
                        nc.tensor.matmul(
                            osl, vw[:, kc, h, :], psl,
                            start=first, stop=last,
                        )
                    else:
                        # two K=64 halves on array tiles (0,0)/(64,0); they can
                        # run concurrently, so they need separate PSUM regions
                        nc.tensor.matmul(
                            osl, vw[0:64, kc, h, :], psl[0:64, :],
                            start=first, stop=last,
                        )
                        nc.tensor.matmul(
                            o_hi[:, nh * 512:(nh + 1) * 512],
                            vw[64:128, kc, h, :], psl[64:128, :],
                            start=first, stop=last,
                        )

            # software pipeline: keep PE one S-matmul ahead of ACT's exp.
            # The previous iteration's evacuate/transpose/normalize tail is
            # emitted after this iteration's first two S matmuls so it
            # overlaps the new exp stream instead of stalling it.
            s_prev = emit_s(0)
            s_cur = emit_s(1)
            for kc in range(1, NKC):
                p_t = emit_exp(kc - 1, s_prev)
                if kc == 2 and pending_tail[0] is not None:
                    pending_tail[0]()
                filler(N_FILLER)
                emit_av(kc - 1, p_t)
                s_prev = s_cur
                s_cur = emit_s(kc + 1) if kc + 1 < NKC else None
            p_t = emit_exp(NKC - 1, s_prev)
            filler(N_FILLER)
            emit_av(NKC - 1, p_t)

            def make_tail(h=h, qh=qh, o_ps=o_ps, o_hi=o_hi):
                def tail():
                    # evacuate O^T, transpose [65,128] blocks -> [128, 65]
                    ot = ot_pool.tile([DH + 1, QH], F32, tag="ot", name=f"ot_{h}_{qh}")
                    if o_hi is not None:
                        nc.vector.tensor_copy(ot[:], o_ps[:])
                        nc.vector.tensor_add(ot[:], ot[:], o_hi[:])
                    else:
                        nc.vector.tensor_copy(ot[:], o_ps[:])
                    tr = psum_s.tile([128, 8, DH + 1], F32, tag="s", name=f"tr_{h}_{qh}")
                    for qb in range(8):
                        nc.tensor.transpose(
                            tr[:, qb, :],
                            ot[:, qb * 128:(qb + 1) * 128],
                            ident[0:DH + 1, 0:DH + 1],
                        )
                    # normalize: out = O * qmask/denom (denom = col 64)
                    rq = rq_pool.tile([128, 8], F32, tag="rq", name=f"rq_{h}_{qh}")
                    nc.vector.reciprocal(rq[:], tr[:, :, DH])
                    nc.vector.tensor_mul(
                        rq[:], rq[:], qmaskT[:, qh * 8:(qh + 1) * 8]
                    )
                    ob = out_pool.tile([128, 8, DH], F32, tag="ob", name=f"ob_{h}_{qh}")
                    nc.vector.tensor_mul(
                        ob[:], tr[:, :, 0:DH], rq[:].broadcast_to([128, 8, DH])
                    )
                    nc.sync.dma_start(out_v[qh][:, :, h, :], ob[:])
                return tail

            pending_tail[0] = make_tail()

    pending_tail[0]()
    actx.close()
    ctx.close()


_BUILD_LOCK = threading.Lock()
_CACHE = {}


def _build():
    with _BUILD_LOCK:
        if "nc" in _CACHE:
            return _CACHE["nc"]
        nc = bacc.Bacc(
            "TRN2", target_bir_lowering=False, debug=False, num_devices=N_CORES
        )
        t = {
            "xq": nc.dram_tensor("xq", [S, D], F32R, kind="ExternalInput"),
            "xk": nc.dram_tensor("xk", [S, D], F32R, kind="ExternalInput"),
            "xv": nc.dram_tensor("xv", [S, D], F32R, kind="ExternalInput"),
            "wq": nc.dram_tensor("wq", [D, MC], F32R, kind="ExternalInput"),
            "wk": nc.dram_tensor("wk", [D, MC], F32R, kind="ExternalInput"),
            "wv": nc.dram_tensor("wv", [D, MC], F32R, kind="ExternalInput"),
            "vbias": nc.dram_tensor("vbias", [128, NKC], F32, kind="ExternalInput"),
            "qmaskT": nc.dram_tensor("qmaskT", [128, NSC], F32, kind="ExternalInput"),
            "out": nc.dram_tensor("out", [S, MC], F32, kind="ExternalOutput"),
        }
        with tile.TileContext(nc) as tc:
            _emit(tc, t)
        nc.compile()
        _CACHE["nc"] = nc
        return nc


def _in_maps(q_value, k_value, v_value, v_mask, q_mask, Wq, Wk, Wv):
    maps = []
    for c in range(N_CORES):
        b, g = c // 2, c % 2
        m0 = g * MC
        vb = ((v_mask[b, :, 0].reshape(NKC, 128).T) - 1.0) * NEG_BIG
        qm = q_mask[b, :, 0].reshape(NSC, 128).T
        maps.append({
            "xq": np.ascontiguousarray(q_value[b]),
            "xk": np.ascontiguousarray(k_value[b]),
            "xv": np.ascontiguousarray(v_value[b]),
            "wq": np.ascontiguousarray(Wq[:, m0:m0 + MC]),
            "wk": np.ascontiguousarray(Wk[:, m0:m0 + MC]),
            "wv": np.ascontiguousarray(Wv[:, m0:m0 + MC]),
            "vbias": np.ascontiguousarray(vb).astype(np.float32),
            "qmaskT": np.ascontiguousarray(qm).astype(np.float32),
        })
    return maps


def _assemble(results):
    out = np.empty((B, S, HEADS * DH), dtype=np.float32)
    for c in range(N_CORES):
        b, g = c // 2, c % 2
        out[b, :, g * MC:(g + 1) * MC] = results[c]["out"]
    return out


def kernel(q_value, k_value, v_value, v_mask, q_mask, Wq, Wk, Wv,
           profile=False, trace_cores=None):
    nc = _build()
    maps = _in_maps(np.asarray(q_value, dtype=np.float32),
                    np.asarray(k_value, dtype=np.float32),
                    np.asarray(v_value, dtype=np.float32),
                    np.asarray(v_mask, dtype=np.float32),
                    np.asarray(q_mask, dtype=np.float32),
                    np.asarray(Wq, dtype=np.float32),
                    np.asarray(Wk, dtype=np.float32),
                    np.asarray(Wv, dtype=np.float32))
    if profile:
        _install_profile_hook()
    res = run_bass_kernel_spmd(
        nc, maps, list(range(N_CORES)),
        trace=profile, trace_cores=trace_cores,
    )
    out = _assemble(res.results)
    if profile:
        return out, res
    return out


def _install_profile_hook():
    """Wire up the NTFF profile hook that this container image lacks."""
    import types
    if "antenv.axon_hooks" in sys.modules:
        return
    try:
        from trn_agent_boot.trn_boot import _ntff_profile_via_ctypes
        hook = _ntff_profile_via_ctypes("/opt/axon/libaxon_pjrt.so")
    except Exception:
        hook = None
    mod = types.ModuleType("antenv.axon_hooks")
    mod.get_axon_ntff_profile_hook = lambda: hook
    sys.modules["antenv.axon_hooks"] = mod


if __name__ == "__main__":
    t0 = time.time()
    _build()
    print(f"build+compile: {time.time() - t0:.1f}s")


# revision 38
# speedup vs baseline: 1.0224x; 1.0224x over previous
"""Trainium2 Bass kernel for batched multi-head attention.

Full module:  out = softmax((X_q Wq)(X_k Wk)^T / sqrt(dh) + keymask) (X_v Wv) * qmask
Shapes: B=4, S=2048, D=1024, H=16, dh=64.

Sharding over 8 NeuronCores: core c -> (batch b = c//2, head-group g = c%2).
Each core computes batch b, heads g*8..g*8+8 (Wq/Wk/Wv column-sharded by head).
No collectives; the host scatters inputs and gathers the [2048, 512] output
blocks into the full [4, 2048, 1024] output.

Per-core dataflow (all matmuls in float32r -> full PE rate at N>=256):
  1. PE-transpose X_q/X_k/X_v tiles to X^T (contraction dim on partitions).
  2. Projections: QW^T,KW^T = (W chunks)^T stationary x X^T moving -> [m, s];
     VW = (X^T chunks) stationary x W moving -> [s, m] (natural), stored with
     a ones-column appended per head for free softmax denominators.
  3. Per head h, q-half qh (softmax-pipelined over 16 k-chunks):
       S^T(kc) = KW^T_chunk^T @ QW^T      (K=64 matmul, auto 64x128 array tile)
       P(kc)   = exp(S^T * 0.125 + vbias) (ScalarE, mask+scale fused)
       O^T    += [VW|1]^T @ P(kc)         (K=128, accumulated in PSUM)
     Then PE-transpose the [65, q] O^T block (row 64 = sum of exp), and
     normalize out = O * (qmask/denom) on VectorE.
"""

import os
import sys
import time
import threading

for _p in ("/opt/trn_rl_repo", "/opt/pypackages"):
    if _p not in sys.path and os.path.isdir(_p):
        sys.path.append(_p)

import numpy as np
from contextlib import ExitStack

import concourse.bass as bass
import concourse.tile as tile
from concourse import bacc, mybir
from concourse.bass_utils import run_bass_kernel_spmd
from concourse.masks import make_identity

B, S, D = 4, 2048, 1024
HEADS, DH = 16, 64
NEG_BIG = 1e10
N_CORES = 8
HG = HEADS // 2          # 8 heads per core
MC = HG * DH             # 512 output cols per core
NSC = S // 128           # 16 seq chunks
NDC = D // 128           # 8 contraction chunks
NMC = MC // 128          # 4 head-dim chunks (of this core's 512 cols)
NKC = NSC                # 16 key chunks
NQH = 2                  # q halves
QH = S // NQH            # 1024

F32 = mybir.dt.float32
F32R = mybir.dt.float32r
EXP = mybir.ActivationFunctionType.Exp

# "k128": AV as one K=128 matmul (array mode switches 64<->128 per k-chunk)
# "k64" : AV split into two K=64 matmuls on array tiles (0,0)/(64,0) -> the
#         whole attention loop stays in 64x128 row-tiled mode.
AV_MODE = os.environ.get("AV_MODE", "k64")
N_FILLER = int(os.environ.get("N_FILLER", "0"))


def _r(ap):
    """reinterpret an fp32 AP as float32r for full-rate PE matmul"""
    return ap.bitcast(F32R)


def _emit(tc, t):
    nc = tc.nc
    ctx = ExitStack()

    # ---------------- persistent pools ----------------
    cpool = ctx.enter_context(tc.tile_pool(name="const", bufs=1))
    # prefetch the first X tiles before anything else so the transpose
    # pipeline starts as early as possible
    xq_dram = t["xq"].ap().rearrange("(sc p) d -> sc p d", p=128)
    pre_pool = ctx.enter_context(tc.tile_pool(name="pre", bufs=1))
    pre_x = []
    for i in range(6):
        xpre = pre_pool.tile([128, D], F32R, name=f"xpre{i}", tag=f"xpre{i}")
        nc.sync.dma_start(xpre[:], xq_dram[i])
        pre_x.append(xpre)

    ident = cpool.tile([128, 128], F32)
    make_identity(nc, ident[:])
    ident_r = cpool.tile([128, 128], F32R)
    nc.vector.tensor_copy(ident_r[:], ident[:])
    vbias = cpool.tile([128, NKC], F32)
    nc.sync.dma_start(vbias[:], t["vbias"].ap())
    qmaskT = cpool.tile([128, NSC], F32)
    nc.sync.dma_start(qmaskT[:], t["qmaskT"].ap())

    scratch_bf = cpool.tile([128, 128], mybir.dt.bfloat16)
    nc.vector.memset(scratch_bf[:], 0.0)

    qk_pool = ctx.enter_context(tc.tile_pool(name="qk", bufs=1))
    qwT = qk_pool.tile([128, NMC, S], F32R)        # [m%128, mc, s] 32KB/part
    kwT = qk_pool.tile([128, NMC, S], F32R)
    vw = qk_pool.tile([128, NKC, HG, DH + 1], F32R)  # [k%128, kc, h, dh|1]
    ones = cpool.tile([128, 1], F32)
    nc.vector.memset(ones[:], 1.0)
    nc.vector.tensor_copy(                           # denominator ones column
        vw[:, :, :, DH:DH + 1], ones[:].broadcast_to([128, NKC, HG, 1])
    )

    # ---------------- projection phase ----------------
    pctx = ExitStack()
    xt_pool = pctx.enter_context(tc.tile_pool(name="xt", bufs=1))
    x_pool = pctx.enter_context(tc.tile_pool(name="x", bufs=4))
    w_pool = pctx.enter_context(tc.tile_pool(name="w", bufs=2))
    psum_t = pctx.enter_context(tc.tile_pool(name="ps_t", bufs=2, space="PSUM"))
    psum_p = pctx.enter_context(tc.tile_pool(name="ps_p", bufs=2, space="PSUM"))

    HSC = NSC // 2  # s-chunks per half

    for xi, (xname, kind) in enumerate((("xq", "q"), ("xk", "k"), ("xv", "v"))):
        x_dram = t[xname].ap().rearrange("(sc p) d -> sc p d", p=128)
        w_dram = t["w" + kind].ap().rearrange("(dc p) m -> p dc m", p=128)
        w_sb = w_pool.tile([128, NDC, MC], F32R, tag="w")
        nc.sync.dma_start(w_sb[:], w_dram)

        for sh in range(2):  # s-halves
            # transpose this half of X into xt [d%128, dc, s_local]
            xt = xt_pool.tile([128, NDC, QH], F32R, tag="xt")
            for scl in range(HSC):
                sc = sh * HSC + scl
                if xname == "xq" and sh == 0 and scl < len(pre_x):
                    xt_in = pre_x[scl]
                else:
                    xt_in = x_pool.tile([128, D], F32R, tag="x")
                    nc.sync.dma_start(xt_in[:], x_dram[sc])
                pt = psum_t.tile([128, NDC, 128], F32R, tag="pt")
                for dc in range(NDC):
                    nc.tensor.transpose(
                        pt[:, dc, :], xt_in[:, dc * 128:(dc + 1) * 128], ident_r[:]
                    )
                if scl % 2 == 0:
                    nc.vector.tensor_copy(xt[:, :, scl * 128:(scl + 1) * 128], pt[:])
                else:
                    nc.scalar.copy(xt[:, :, scl * 128:(scl + 1) * 128], pt[:])

            if kind in ("q", "k"):
                dst = qwT if kind == "q" else kwT
                for mc in range(NMC):
                    pp = psum_p.tile([128, QH], F32, tag="pp")
                    for dc in range(NDC):
                        for nh in range(QH // 512):
                            nc.tensor.matmul(
                                pp[:, nh * 512:(nh + 1) * 512],
                                w_sb[:, dc, mc * 128:(mc + 1) * 128],
                                xt[:, dc, nh * 512:(nh + 1) * 512],
                                start=(dc == 0),
                                stop=(dc == NDC - 1),
                            )
                    nc.vector.tensor_copy(
                        dst[:, mc, sh * QH:(sh + 1) * QH], pp[:]
                    )
            else:
                for scl in range(HSC):
                    sc = sh * HSC + scl
                    pv = psum_p.tile([128, MC], F32, tag="pp")
                    for dc in range(NDC):
                        nc.tensor.matmul(
                            pv[:],
                            xt[:, dc, scl * 128:(scl + 1) * 128],
                            w_sb[:, dc, :],
                            start=(dc == 0),
                            stop=(dc == NDC - 1),
                        )
                    nc.vector.tensor_copy(
                        vw[:, sc, :, 0:DH],
                        pv[:].rearrange("p (h d) -> p h d", h=HG),
                    )

    pctx.close()

    # ---------------- attention phase ----------------
    actx = ExitStack()
    p_pool = actx.enter_context(tc.tile_pool(name="p", bufs=3))
    ot_pool = actx.enter_context(tc.tile_pool(name="ot", bufs=2))
    rq_pool = actx.enter_context(tc.tile_pool(name="rq", bufs=2))
    out_pool = actx.enter_context(tc.tile_pool(name="out", bufs=3))
    psum_s = actx.enter_context(tc.tile_pool(name="ps_s", bufs=2, space="PSUM"))
    psum_o = actx.enter_context(tc.tile_pool(name="ps_o", bufs=2, space="PSUM"))

    # DRAM view: [qh, p, qb, h, d] for per-(head, q-half) strip stores
    out_v = t["out"].ap().rearrange(
        "(a qb p) (hh d) -> a p qb hh d", a=NQH, p=128, hh=HG
    )

    def filler(n):
        for _ in range(n):
            nc.tensor.ldweights(scratch_bf[:])

    pending_tail = [None]

    for h in range(HG):
        mc_h = h // 2
        p0 = (h % 2) * 64
        kw_h = kwT[p0:p0 + 64, mc_h, :]
        qw_h = qwT[p0:p0 + 64, mc_h, :]
        for qh in range(NQH):
            q0 = qh * QH
            o_ps = psum_o.tile([DH + 1, QH], F32, tag="o")
            if AV_MODE == "k64":
                o_hi = psum_o.tile([DH + 1, QH], F32, tag="o")
            else:
                o_hi = None

            def emit_s(kc):
                s_ps = psum_s.tile([128, QH], F32, tag="s")
                for nh in range(QH // 512):
                    nc.tensor.matmul(
                        s_ps[:, nh * 512:(nh + 1) * 512],
                        kw_h[:, kc * 128:(kc + 1) * 128],
                        qw_h[:, q0 + nh * 512:q0 + (nh + 1) * 512],
                        start=True,
                        stop=True,
                    )
                return s_ps

            def emit_exp(kc, s_ps):
                p_t = p_pool.tile([128, QH], F32R, tag="p")
                nc.scalar.activation(
                    p_t[:], s_ps[:], EXP,
                    bias=vbias[:, kc:kc + 1], scale=0.125,
                )
                return p_t

            def emit_av(kc, p_t):
                first, last = kc == 0, kc == NKC - 1
                for nh in range(QH // 512):
                    osl = o_ps[:, nh * 512:(nh + 1) * 512]
                    psl = p_t[:, nh * 512:(nh + 1) * 512]
                    if AV_MODE == "k128":
                        nc.tensor.matmul(
                            osl, vw[:, kc, h, :], psl,
                            start=first, stop=last,
                        )
                    else:
                        # two K=64 halves on array tiles (0,0)/(64,0); they can
                        # run concurrently, so they need separate PSUM regions
                        nc.tensor.matmul(
                            osl, vw[0:64, kc, h, :], psl[0:64, :],
                            start=first, stop=last,
                        )
                        nc.tensor.matmul(
                            o_hi[:, nh * 512:(nh + 1) * 512],
                            vw[64:128, kc, h, :], psl[64:128, :],
                            start=first, stop=last,
                        )

            # software pipeline: keep PE one S-matmul ahead of ACT's exp.
            # The previous iteration's evacuate/transpose/normalize tail is
            # emitted after this iteration's first two S matmuls so it
            # overlaps the new exp stream instead of stalling it.
            s_prev = emit_s(0)
            s_cur = emit_s(1)
            for kc in range(1, NKC):
                p_t = emit_exp(kc - 1, s_prev)
                if kc == 2 and pending_tail[0] is not None:
                    pending_tail[0]()
                filler(N_FILLER)
                emit_av(kc - 1, p_t)
                s_prev = s_cur
                s_cur = emit_s(kc + 1) if kc + 1 < NKC else None
            p_t = emit_exp(NKC - 1, s_prev)
            filler(N_FILLER)
            emit_av(NKC - 1, p_t)

            def make_tail(h=h, qh=qh, o_ps=o_ps, o_hi=o_hi):
                def tail():
                    # evacuate O^T, transpose [65,128] blocks -> [128, 65]
                    ot = ot_pool.tile([DH + 1, QH], F32, tag="ot", name=f"ot_{h}_{qh}")
                    if o_hi is not None:
                        nc.vector.tensor_copy(ot[:], o_ps[:])
                        nc.vector.tensor_add(ot[:], ot[:], o_hi[:])
                    else:
                        nc.vector.tensor_copy(ot[:], o_ps[:])
                    tr = psum_s.tile([128, 8, DH + 1], F32, tag="s", name=f"tr_{h}_{qh}")
                    for qb in range(8):
                        nc.tensor.transpose(
                            tr[:, qb, :],
                            ot[:, qb * 128:(qb + 1) * 128],
                            ident[0:DH + 1, 0:DH + 1],
                        )
                    # normalize: out = O * qmask/denom (denom = col 64)
                    rq = rq_pool.tile([128, 8], F32, tag="rq", name=f"rq_{h}_{qh}")
                    nc.vector.reciprocal(rq[:], tr[:, :, DH])
                    nc.vector.tensor_mul(
                        rq[:], rq[:], qmaskT[:, qh * 8:(qh + 1) * 8]
                    )
                    ob = out_pool.tile([128, 8, DH], F32, tag="ob", name=f"ob_{h}_{qh}")
                    nc.vector.tensor_mul(
                        ob[:], tr[:, :, 0:DH], rq[:].broadcast_to([128, 8, DH])
                    )
                    nc.sync.dma_start(out_v[qh][:, :, h, :], ob[:])
                return tail

            pending_tail[0] = make_tail()

    pending_tail[0]()
    actx.close()
    ctx.close()


_BUILD_LOCK = threading.Lock()
_CACHE = {}


def _build():
    with _BUILD_LOCK:
        if "nc" in _CACHE:
            return _CACHE["nc"]
        nc = bacc.Bacc(
            "TRN2", target_bir_lowering=False, debug=False, num_devices=N_CORES
        )
        t = {
            "xq": nc.dram_tensor("xq", [S, D], F32R, kind="ExternalInput"),
            "xk": nc.dram_tensor("xk", [S, D], F32R, kind="ExternalInput"),
            "xv": nc.dram_tensor("xv", [S, D], F32R, kind="ExternalInput"),
            "wq": nc.dram_tensor("wq", [D, MC], F32R, kind="ExternalInput"),
            "wk": nc.dram_tensor("wk", [D, MC], F32R, kind="ExternalInput"),
            "wv": nc.dram_tensor("wv", [D, MC], F32R, kind="ExternalInput"),
            "vbias": nc.dram_tensor("vbias", [128, NKC], F32, kind="ExternalInput"),
            "qmaskT": nc.dram_tensor("qmaskT", [128, NSC], F32, kind="ExternalInput"),
            "out": nc.dram_tensor("out", [S, MC], F32, kind="ExternalOutput"),
        }
        with tile.TileContext(nc) as tc:
            _emit(tc, t)
        nc.compile()
        _CACHE["nc"] = nc
        return nc


def _in_maps(q_value, k_value, v_value, v_mask, q_mask, Wq, Wk, Wv):
    maps = []
    for c in range(N_CORES):
        b, g = c // 2, c % 2
        m0 = g * MC
        vb = ((v_mask[b, :, 0].reshape(NKC, 128).T) - 1.0) * NEG_BIG
        qm = q_mask[b, :, 0].reshape(NSC, 128).T
        maps.append({
            "xq": np.ascontiguousarray(q_value[b]),
            "xk": np.ascontiguousarray(k_value[b]),
            "xv": np.ascontiguousarray(v_value[b]),
            "wq": np.ascontiguousarray(Wq[:, m0:m0 + MC]),
            "wk": np.ascontiguousarray(Wk[:, m0:m0 + MC]),
            "wv": np.ascontiguousarray(Wv[:, m0:m0 + MC]),
            "vbias": np.ascontiguousarray(vb).astype(np.float32),
            "qmaskT": np.ascontiguousarray(qm).astype(np.float32),
        })
    return maps


def _assemble(results):
    out = np.empty((B, S, HEADS * DH), dtype=np.float32)
    for c in range(N_CORES):
        b, g = c // 2, c % 2
        out[b, :, g * MC:(g + 1) * MC] = results[c]["out"]
    return out


def kernel(q_value, k_value, v_value, v_mask, q_mask, Wq, Wk, Wv,
           profile=False, trace_cores=None):
    nc = _build()
    maps = _in_maps(np.asarray(q_value, dtype=np.float32),
                    np.asarray(k_value, dtype=np.float32),
                    np.asarray(v_value, dtype=np.float32),
                    np.asarray(v_mask, dtype=np.float32),
                    np.asarray(q_mask, dtype=np.float32),
                    np.asarray(Wq, dtype=np.float32),
                    np.asarray(Wk, dtype=np.float32),
                    np.asarray(Wv, dtype=np.float32))
    if profile:
        _install_profile_hook()
    res = run_bass_kernel_spmd(
        nc, maps, list(range(N_CORES)),
        trace=profile, trace_cores=trace_cores,
    )
    out = _assemble(res.results)
    if profile:
        return out, res
    return out


def _install_profile_hook():
    """Wire up the NTFF profile hook that this container image lacks."""
    import types
    if "antenv.axon_hooks" in sys.modules:
        return
    try:
        from trn_agent_boot.trn_boot import _ntff_profile_via_ctypes
        hook = _ntff_profile_via_ctypes("/opt/axon/libaxon_pjrt.so")
    except Exception:
        hook = None
    mod = types.ModuleType("antenv.axon_hooks")
    mod.get_axon_ntff_profile_hook = lambda: hook
    sys.modules["antenv.axon_hooks"] = mod


if __name__ == "__main__":
    t0 = time.time()
    _build()
    print(f"build+compile: {time.time() - t0:.1f}s")
